# revision 33
# baseline (speedup 1.0000x reference)
"""Trainium2 Bass kernel for nn_MetaLearner (meta-learning attention + cosine
prototype scoring), data-parallel over tasks on 8 NeuronCores.

Math (per task):
  c   = [img, txt] @ Wc.T + bc                (Wc = concat(Wi, Wt))
  h   = LN1(c);  q,k,v = h @ W{q,k,v}.T + b   (queries: seqlen=1 -> ctx = v)
  ctx = softmax(q k^T / sqrt(128)) v          (support: seqlen=4)
  f   = LN2(ctx) @ Wo.T + bo
  logits[t,q,c] = 10 * cos(qf[t,q], sf[t,c])

Key tricks:
  - LN gains/biases and mean-subtractions folded into weights on host.
  - Softmax denominator (and max-subtraction) dropped: LN2's rstd cancels
    any positive per-column scale of ctx; mean-centering folded into Wv.
  - When bc/bv/bo fold to zero (true for the reference initialization),
    every per-column scale cancels through the final cosine normalize, so
    the whole query path collapses to qf = normalize(Wu x) with
    Wu = (Wo Wv_c) Wc_c folded on host -- queries never materialize c, h,
    v, or f; the projection accumulates directly during streaming.
    Support keeps LN1 (softmax is scale-sensitive) but skips LN2's rstd.
    A general fallback path keeps the full math.
  - rstd = Sqrt(reciprocal_approx_fast(scale * colsum(x^2))): the scale
    rides in the reduction weights, DVE does the reciprocal, ACT only ever
    evaluates Sqrt (plus one Exp for softmax) -> ~2 table switches total.
  - No transposes: attention scores computed pre-transposed (sT = kT^T qT)
    and v computed pre-transposed (vn = h^T WvT) by operand swapping.
  - All PE operands fp16 (1 cyc/row); f32 accumulation in PSUM.
  - Inputs streamed as fp16, host-packed per column-group so every stream
    DMA is one contiguous 1.4-5.6 MB transfer. The last group is only 256
    columns so the final (unhidable) tail chain is short.
On-chip layout is "transposed" throughout: activations are [hid, rows].
"""
import sys
sys.path.insert(0, "/opt/trn_rl_repo")
import numpy as np

HID = 128
T, Q, S = 256, 64, 4
DI, DTXT = 2048, 768
NCORES = 8
TPC = T // NCORES               # 32 tasks per core
FEAT = DI + DTXT                # 2816
KT = FEAT // 128                # 22 contraction chunks
QROWS = TPC * Q                 # 2048 query rows per core
SROWS = TPC * S                 # 128 support rows per core
ROWS = QROWS + SROWS            # 2176
SCALE_INV = 1.0 / (np.sqrt(HID) + 1e-8)

_progs = {}  # cached compiled Bass programs, keyed by fast-path flag


def _build(fast):
    import concourse.bacc as bacc
    import concourse.tile as tile
    import concourse.mybir as mybir
    import concourse.bass as _b

    F32 = mybir.dt.float32
    F16 = mybir.dt.float16
    AFT = mybir.ActivationFunctionType

    nc = bacc.Bacc()
    x1_d = nc.declare_dram_parameter("x1", [128, KT * 1152], F16, isOutput=False)
    x2_d = nc.declare_dram_parameter("x2", [128, KT * 512], F16, isOutput=False)
    x3a_d = nc.declare_dram_parameter("x3a", [128, KT * 320], F16,
                                      isOutput=False)
    x3b_d = nc.declare_dram_parameter("x3b", [128, KT * 192], F16,
                                      isOutput=False)
    wcwu_d = nc.declare_dram_parameter("wcwu", [128, KT * 256], F16,
                                       isOutput=False)
    cst_d = nc.declare_dram_parameter("consts", [128, 772], F16, isOutput=False)
    row0_d = nc.declare_dram_parameter("row0", [1, 768], F16, isOutput=False)
    bias_d = (None if fast else
              nc.declare_dram_parameter("biases", [HID, 5], F32,
                                        isOutput=False))
    out_d = nc.declare_dram_parameter("logits", [TPC, Q, S], F32, isOutput=True)

    lp = nc.allow_low_precision(reason="fp16 streaming with f32 accumulation")
    lp.__enter__()

    with tile.TileContext(nc) as tc:
        with (
            tc.tile_pool(name="wts", bufs=1) as wts,
            tc.tile_pool(name="xg1p", bufs=1) as xg1p,
            tc.tile_pool(name="xg2p", bufs=1) as xg2p,
            tc.tile_pool(name="xg3p", bufs=1) as xg3p,
            tc.tile_pool(name="qfp", bufs=1) as qfp,
            tc.tile_pool(name="wk", bufs=3) as wk,
            tc.tile_pool(name="pst", bufs=1, space="PSUM") as pst,
        ):
            # ---- loads, in stream order ----
            wcwu_t = wts.tile([128, KT, 256], F16)
            nc.sync.dma_start(out=wcwu_t, in_=wcwu_d[:])
            xg1_t = xg1p.tile([128, KT, 1152], F16)
            nc.sync.dma_start(out=xg1_t[:, 0:8, :], in_=x1_d[:, 0:8 * 1152])
            nc.sync.dma_start(out=xg1_t[:, 8:15, :],
                              in_=x1_d[:, 8 * 1152:15 * 1152])
            nc.sync.dma_start(out=xg1_t[:, 15:KT, :], in_=x1_d[:, 15 * 1152:])
            cst_t = wts.tile([128, 772], F16)
            nc.sync.dma_start(out=cst_t, in_=cst_d[:])
            wq_t = cst_t[:, 0 * HID:1 * HID]
            wk_t = cst_t[:, 1 * HID:2 * HID]
            wv_t = cst_t[:, 2 * HID:3 * HID]
            wo_t = cst_t[:, 3 * HID:4 * HID]
            wov_t = cst_t[:, 4 * HID:5 * HID]
            red_ln = cst_t[:, 640:641]           # 1/128
            red_q = cst_t[:, 641:642]            # 1.0
            red_s = cst_t[:, 642:643]            # 0.01
            mask_t = cst_t[:, 644:772]           # [128, 128] 0/1 block mask
            row0_t = wts.tile([1, 768], F16)
            nc.sync.dma_start(out=row0_t, in_=row0_d[:])
            ones_r = row0_t[:, 0:128]            # lhsT [K=1, M=128] for bcast
            ones_row = row0_t[:, 0:640]          # all-ones rhs
            bvrow_t = row0_t[:, 640:768]
            if not fast:
                bias_t = wts.tile([HID, 5], F32)
                nc.sync.dma_start(out=bias_t, in_=bias_d[:])
                bc_a, bq_a, bk_a, bv_a, bo_a = (
                    bias_t[:, i:i + 1] for i in range(5))
            else:
                bc_a = bq_a = bk_a = bv_a = bo_a = None
            xg2_t = xg2p.tile([128, KT, 512], F16)
            nc.sync.dma_start(out=xg2_t[:, 0:11, :], in_=x2_d[:, 0:11 * 512])
            nc.sync.dma_start(out=xg2_t[:, 11:KT, :], in_=x2_d[:, 11 * 512:])
            xg3a_t = xg3p.tile([128, KT, 320], F16)
            nc.sync.dma_start(out=xg3a_t[:, 0:11, :], in_=x3a_d[:, 0:11 * 320])
            nc.sync.dma_start(out=xg3a_t[:, 11:KT, :], in_=x3a_d[:, 11 * 320:])
            xg3b_t = xg3p.tile([128, KT, 192], F16)
            nc.sync.dma_start(out=xg3b_t[:, 0:11, :], in_=x3b_d[:, 0:11 * 192])
            nc.sync.dma_start(out=xg3b_t[:, 11:18, :],
                              in_=x3b_d[:, 11 * 192:18 * 192])
            nc.sync.dma_start(out=xg3b_t[:, 18:KT, :], in_=x3b_d[:, 18 * 192:])

            qf_t = qfp.tile([128, QROWS], F16)
            sf_t = qfp.tile([128, SROWS], F16)
            U_sb = qfp.tile([64, 2 * Q], F32)

            def rstd(sq_sb, cn, red):
                """1/sqrt(red . sq) broadcast to [128, cn] PSUM (f16 path)."""
                ss_ps = pst.tile([1, 512], F32, tag="ss", bufs=2)
                nc.tensor.matmul(ss_ps[:, :cn], red, sq_sb[:, :cn],
                                 start=True, stop=True)
                ir = wk.tile([1, 512], F32, tag="ir")
                nc.vector.reciprocal_approx_fast(out=ir[:, :cn],
                                                 in_=ss_ps[:, :cn])
                rr = wk.tile([1, 512], F16, tag="rr")
                nc.scalar.activation(out=rr[:, :cn], in_=ir[:, :cn],
                                     func=AFT.Sqrt, bias=0.0, scale=1.0)
                R_ps = pst.tile([128, 512], F32, tag="pp", bufs=2)
                nc.tensor.matmul(R_ps[:, :cn], ones_r, rr[:, :cn],
                                 start=True, stop=True)
                return R_ps

            def score(t0, nt):
                """U_sb[q, 4t:4(t+nt)] = qf[:,64t:...]^T sf[:,4t:...] x nt."""
                U_ps = pst.tile([64, 32], F32, tag="sc", bufs=1)
                for j in range(nt):
                    t = t0 + j
                    nc.tensor.matmul(U_ps[0:64, 4 * j:4 * j + 4],
                                     qf_t[:, 64 * t:64 * t + 64],
                                     sf_t[:, 4 * t:4 * t + 4],
                                     start=True, stop=True)
                nc.vector.tensor_copy(
                    out=U_sb[0:64, 4 * t0:4 * (t0 + nt)],
                    in_=U_ps[0:64, 0:4 * nt])

            def query_tail_fast(u_ps, qf_off, cn):
                """qf = normalize(u); u = Wu x accumulated during streaming."""
                ff = wk.tile([128, 512], F16, tag="ff")
                nc.vector.tensor_copy(out=ff[:, :cn], in_=u_ps[:, :cn])
                sq = wk.tile([128, 512], F16, tag="sq")
                nc.scalar.activation(out=sq[:, :cn], in_=u_ps[:, :cn],
                                     func=AFT.Square, bias=0.0, scale=1.0)
                RN = rstd(sq, cn, red_q)
                if cn <= 320:
                    # per-task pieces: scoring LDW_j starts after its slice
                    for j in range(cn // 64):
                        nc.vector.tensor_mul(
                            out=qf_t[:, qf_off + 64 * j:qf_off + 64 * j + 64],
                            in0=ff[:, 64 * j:64 * j + 64],
                            in1=RN[:, 64 * j:64 * j + 64])
                else:
                    nc.vector.tensor_mul(out=qf_t[:, qf_off:qf_off + cn],
                                         in0=ff[:, :cn], in1=RN[:, :cn])
                score(qf_off // 64, cn // 64)

            def query_tail_gen(c_ps, qf_off, cn):
                cf = wk.tile([128, 512], F16, tag="cf")
                nc.vector.tensor_scalar_add(out=cf[:, :cn], in0=c_ps[:, :cn],
                                            scalar1=bc_a)
                sq = wk.tile([128, 512], F16, tag="sq")
                nc.scalar.activation(out=sq[:, :cn], in_=c_ps[:, :cn],
                                     func=AFT.Square, bias=bc_a, scale=1.0)
                R1 = rstd(sq, cn, red_ln)
                h = wk.tile([128, 512], F16, tag="h")
                nc.vector.tensor_mul(out=h[:, :cn], in0=cf[:, :cn],
                                     in1=R1[:, :cn])
                v_ps = pst.tile([128, 512], F32, tag="pp", bufs=2)
                nc.tensor.matmul(v_ps[:, :cn], wv_t, h[:, :cn],
                                 start=True, stop=True)
                vf = wk.tile([128, 512], F16, tag="vf")
                nc.vector.tensor_scalar_add(out=vf[:, :cn], in0=v_ps[:, :cn],
                                            scalar1=bv_a)
                sq2 = wk.tile([128, 512], F16, tag="sq")
                nc.scalar.activation(out=sq2[:, :cn], in_=v_ps[:, :cn],
                                     func=AFT.Square, bias=bv_a, scale=1.0)
                R2 = rstd(sq2, cn, red_ln)
                z = wk.tile([128, 512], F16, tag="h")
                nc.vector.tensor_mul(out=z[:, :cn], in0=vf[:, :cn],
                                     in1=R2[:, :cn])
                o_ps = pst.tile([128, 512], F32, tag="pp", bufs=2)
                nc.tensor.matmul(o_ps[:, :cn], wo_t, z[:, :cn],
                                 start=True, stop=True)
                ff = wk.tile([128, 512], F16, tag="ff")
                nc.vector.tensor_scalar_add(out=ff[:, :cn], in0=o_ps[:, :cn],
                                            scalar1=bo_a)
                sq3 = wk.tile([128, 512], F16, tag="sq")
                nc.scalar.activation(out=sq3[:, :cn], in_=o_ps[:, :cn],
                                     func=AFT.Square, bias=bo_a, scale=1.0)
                RN = rstd(sq3, cn, red_q)
                nc.vector.tensor_mul(out=qf_t[:, qf_off:qf_off + cn],
                                     in0=ff[:, :cn], in1=RN[:, :cn])
                score(qf_off // 64, cn // 64)

            query_tail = query_tail_fast if fast else query_tail_gen

            def support_tail(cS_ps):
                cfS = wk.tile([128, SROWS], F16, tag="cfS")
                if fast:
                    nc.vector.tensor_copy(out=cfS, in_=cS_ps[:, 0:SROWS])
                else:
                    nc.vector.tensor_scalar_add(out=cfS, in0=cS_ps[:, 0:SROWS],
                                                scalar1=bc_a)
                sqS = wk.tile([128, SROWS], F16, tag="sqS")
                if fast:
                    nc.scalar.activation(out=sqS, in_=cS_ps[:, 0:SROWS],
                                         func=AFT.Square, bias=0.0, scale=1.0)
                else:
                    nc.scalar.activation(out=sqS, in_=cS_ps[:, 0:SROWS],
                                         func=AFT.Square, bias=bc_a, scale=1.0)
                R1 = rstd(sqS, SROWS, red_ln)
                hs = wk.tile([128, SROWS], F16, tag="hs")
                nc.vector.tensor_mul(out=hs, in0=cfS, in1=R1[:, :SROWS])
                q_ps = pst.tile([128, 512], F32, tag="pp", bufs=2)
                nc.tensor.matmul(q_ps[:, :SROWS], wq_t, hs, start=True, stop=True)
                qTb = wk.tile([128, SROWS], F16, tag="qTb")
                if fast:
                    nc.vector.tensor_copy(out=qTb, in_=q_ps[:, :SROWS])
                else:
                    nc.vector.tensor_scalar_add(out=qTb, in0=q_ps[:, :SROWS],
                                                scalar1=bq_a)
                k_ps = pst.tile([128, 512], F32, tag="pp", bufs=2)
                nc.tensor.matmul(k_ps[:, :SROWS], wk_t, hs, start=True, stop=True)
                kTb = wk.tile([128, SROWS], F16, tag="kTb")
                if fast:
                    nc.vector.tensor_copy(out=kTb, in_=k_ps[:, :SROWS])
                else:
                    nc.vector.tensor_scalar_add(out=kTb, in0=k_ps[:, :SROWS],
                                                scalar1=bk_a)
                # vn = (Wv h)^T (+ 1 (x) bv in the general path) : [srows, hid]
                vn_ps = pst.tile([128, 512], F32, tag="pp", bufs=2)
                nc.tensor.matmul(vn_ps[:, :HID], hs, wv_t, start=True,
                                 stop=fast)
                if not fast:
                    nc.tensor.matmul(vn_ps[:, :HID], ones_row[:, 0:SROWS],
                                     bvrow_t, start=False, stop=True)
                vn_sb = wk.tile([SROWS, HID], F16, tag="vn")
                nc.vector.tensor_copy(out=vn_sb, in_=vn_ps[:, :HID])
                # sT[k_row, q_row] = k^T q ; unnormalized masked exp
                sT_ps = pst.tile([128, 512], F32, tag="pp", bufs=2)
                nc.tensor.matmul(sT_ps[:, :SROWS], kTb, qTb, start=True,
                                 stop=True)
                aT = wk.tile([SROWS, SROWS], F16, tag="aT")
                nc.scalar.activation(out=aT, in_=sT_ps[:, :SROWS], func=AFT.Exp,
                                     bias=0.0, scale=1.0)
                am = wk.tile([SROWS, SROWS], F16, tag="am")
                nc.vector.tensor_mul(out=am, in0=aT, in1=mask_t)
                ctx_ps = pst.tile([128, 512], F32, tag="pp", bufs=2)
                nc.tensor.matmul(ctx_ps[:, :SROWS], vn_sb, am, start=True,
                                 stop=True)
                cfx = wk.tile([128, SROWS], F16, tag="cfS")
                nc.vector.tensor_copy(out=cfx, in_=ctx_ps[:, :SROWS])
                if fast:
                    zx = cfx           # LN2 rstd cancels in the final normalize
                else:
                    sqx = wk.tile([128, SROWS], F16, tag="sqS")
                    nc.scalar.activation(out=sqx, in_=ctx_ps[:, :SROWS],
                                         func=AFT.Square, bias=0.0, scale=1.0)
                    R2 = rstd(sqx, SROWS, red_ln)
                    zx = wk.tile([128, SROWS], F16, tag="hs")
                    nc.vector.tensor_mul(out=zx, in0=cfx, in1=R2[:, :SROWS])
                o_ps = pst.tile([128, 512], F32, tag="pp", bufs=2)
                nc.tensor.matmul(o_ps[:, :SROWS], wo_t, zx, start=True, stop=True)
                ffx = wk.tile([128, SROWS], F16, tag="ffS")
                if fast:
                    nc.vector.tensor_copy(out=ffx, in_=o_ps[:, :SROWS])
                else:
                    nc.vector.tensor_scalar_add(out=ffx, in0=o_ps[:, :SROWS],
                                                scalar1=bo_a)
                sqf = wk.tile([128, SROWS], F16, tag="sqS")
                if fast:
                    nc.scalar.activation(out=sqf, in_=o_ps[:, :SROWS],
                                         func=AFT.Square, bias=0.0, scale=1.0)
                else:
                    nc.scalar.activation(out=sqf, in_=o_ps[:, :SROWS],
                                         func=AFT.Square, bias=bo_a, scale=1.0)
                RN = rstd(sqf, SROWS, red_s)   # = 10/||f||
                nc.vector.tensor_mul(out=sf_t[:], in0=ffx, in1=RN[:, :SROWS])

            def qw_k(k):
                return (wcwu_t[:, k, 128:256] if fast
                        else wcwu_t[:, k, 0:128])

            def out_dma(t0, nt):
                dst = _b.AP(tensor=out_d[:].tensor,
                            offset=out_d[:].offset + 256 * t0,
                            ap=[[4, 64], [256, nt], [1, 4]])
                nc.sync.dma_start(
                    out=dst,
                    in_=U_sb[0:64, 4 * t0:4 * (t0 + nt)].rearrange(
                        "p (g b) -> p g b", b=4))

            # ---- group 1: support + q0 + q1 ----
            with tc.tile_pool(name="psA", bufs=1, space="PSUM") as psA:
                cS_ps = psA.tile([128, SROWS], F32)
                u0_ps = psA.tile([128, 512], F32)
                u1_ps = psA.tile([128, 512], F32)
                for k in range(KT):
                    nc.tensor.matmul(cS_ps[:], wcwu_t[:, k, 0:128],
                                     xg1_t[:, k, 0:128],
                                     start=(k == 0), stop=(k == KT - 1))
                    nc.tensor.matmul(u0_ps[:], qw_k(k),
                                     xg1_t[:, k, 128:640],
                                     start=(k == 0), stop=(k == KT - 1))
                    nc.tensor.matmul(u1_ps[:], qw_k(k),
                                     xg1_t[:, k, 640:1152],
                                     start=(k == 0), stop=(k == KT - 1))
                support_tail(cS_ps)
                query_tail(u0_ps, 0, 512)
                query_tail(u1_ps, 512, 512)
                out_dma(0, 16)

            # ---- groups 2+3: all stream matmuls back-to-back on PE,
            # then the latency-chained tails (keeps HAM warm at stream end)
            with tc.tile_pool(name="psB", bufs=1, space="PSUM") as psB:
                u2_ps = psB.tile([128, 512], F32)
                for k in range(KT):
                    nc.tensor.matmul(u2_ps[:], qw_k(k), xg2_t[:, k, :],
                                     start=(k == 0), stop=(k == KT - 1))
                with tc.tile_pool(name="psC", bufs=1, space="PSUM") as psC:
                    ua_ps = psC.tile([128, 320], F32)
                    ub_ps = psC.tile([128, 192], F32)
                    for k in range(KT):
                        nc.tensor.matmul(ua_ps[:], qw_k(k), xg3a_t[:, k, :],
                                         start=(k == 0), stop=(k == KT - 1))
                    for k in range(KT):
                        nc.tensor.matmul(ub_ps[:], qw_k(k), xg3b_t[:, k, :],
                                         start=(k == 0), stop=(k == KT - 1))
                    query_tail(u2_ps, 1024, 512)
                    out_dma(16, 8)
                    query_tail(ua_ps, 1536, 320)
                    out_dma(24, 5)
                    query_tail(ub_ps, 1856, 192)
                    out_dma(29, 3)

    lp.__exit__(None, None, None)
    nc.compile()
    return nc


def _host_prep(inputs):
    f32, f16 = np.float32, np.float16
    Wi, Wt = np.asarray(inputs["Wi"], f32), np.asarray(inputs["Wt"], f32)
    bi, bt = np.asarray(inputs["bi"], f32), np.asarray(inputs["bt"], f32)
    g1, b1 = np.asarray(inputs["g1"], f32), np.asarray(inputs["b1"], f32)
    g2, b2 = np.asarray(inputs["g2"], f32), np.asarray(inputs["b2"], f32)
    Wq, bq = np.asarray(inputs["Wq"], f32), np.asarray(inputs["bq"], f32)
    Wk, bk = np.asarray(inputs["Wk"], f32), np.asarray(inputs["bk"], f32)
    Wv, bv = np.asarray(inputs["Wv"], f32), np.asarray(inputs["bv"], f32)
    Wo, bo = np.asarray(inputs["Wo"], f32), np.asarray(inputs["bo"], f32)

    Wc = np.concatenate([Wi, Wt], axis=1)          # [128, 2816]
    bc = bi + bt
    Wc_c = Wc - Wc.mean(axis=0, keepdims=True)     # fold LN1 mean
    bc_c = bc - bc.mean()

    Wq_f = (Wq * g1[None, :]) * SCALE_INV
    bq_f = (bq + Wq @ b1) * SCALE_INV
    Wk_f = Wk * g1[None, :]
    bk_f = bk + Wk @ b1
    Wv_f = Wv * g1[None, :]
    bv_f = bv + Wv @ b1
    Wv_c = Wv_f - Wv_f.mean(axis=0, keepdims=True)  # fold LN2 mean
    bv_c = bv_f - bv_f.mean()
    Wo_f = Wo * g2[None, :]
    bo_f = bo + Wo @ b2
    Wov = Wo_f @ Wv_c                               # combined v+o projection
    Wu = Wov @ Wc_c                                 # full query-path fold

    fast = all(np.abs(b).max() < 1e-12
               for b in (bc_c, bv_c, bo_f, bq_f, bk_f))

    blk = np.arange(SROWS) // S
    mask01 = (blk[:, None] == blk[None, :]).astype(f16)
    consts = np.zeros((128, 772), f16)
    consts[:, 0:640] = np.concatenate(
        [Wq_f.T, Wk_f.T, Wv_c.T, Wo_f.T, Wov.T], axis=1).astype(f16)
    consts[:, 640] = f16(1.0 / HID)
    consts[:, 641] = f16(1.0)
    consts[:, 642] = f16(0.01)
    consts[:, 644:772] = mask01
    row0 = np.zeros((1, 768), f16)
    row0[0, 0:640] = 1.0
    row0[0, 640:768] = bv_c.astype(f16)

    def pack_kmajor(a):   # [feat, cols] -> [128, KT*cols] (p, k, c)
        cols = a.shape[1]
        return np.ascontiguousarray(
            a.reshape(KT, 128, cols).transpose(1, 0, 2).reshape(128, -1)
        )

    wcwu = np.concatenate([Wc_c.T.astype(f16).reshape(KT, 128, 128),
                           Wu.T.astype(f16).reshape(KT, 128, 128)],
                          axis=2)          # [KT, 128, 256] (wc_k | wu_k)
    common = {
        "wcwu": np.ascontiguousarray(
            wcwu.transpose(1, 0, 2).reshape(128, -1)),
        "consts": consts,
        "row0": row0,
    }
    if not fast:
        common["biases"] = np.ascontiguousarray(
            np.stack([bc_c, bq_f, bk_f, bv_c, bo_f], axis=1))

    si = np.asarray(inputs["support_images"], f32)
    st = np.asarray(inputs["support_texts"], f32)
    qi = np.asarray(inputs["query_images"], f32)
    qt = np.asarray(inputs["query_texts"], f32)

    in_maps = []
    for m in range(NCORES):
        ts = slice(m * TPC, (m + 1) * TPC)
        Xq = np.concatenate([qi[ts].reshape(QROWS, DI),
                             qt[ts].reshape(QROWS, DTXT)], axis=1)
        Xs = np.concatenate([si[ts].reshape(SROWS, DI),
                             st[ts].reshape(SROWS, DTXT)], axis=1)
        xT = np.concatenate([Xs, Xq], axis=0).T.astype(f16)  # [2816, 2176]
        in_maps.append({
            "x1": pack_kmajor(xT[:, 0:1152]),
            "x2": pack_kmajor(xT[:, 1152:1664]),
            "x3a": pack_kmajor(xT[:, 1664:1984]),
            "x3b": pack_kmajor(xT[:, 1984:2176]),
            **common,
        })
    return in_maps, fast


def _run(in_maps, fast, trace=False, **kw):
    from concourse.bass_utils import run_bass_kernel_spmd
    if fast not in _progs:
        _progs[fast] = _build(fast)
    return run_bass_kernel_spmd(_progs[fast], in_maps, list(range(NCORES)),
                                trace=trace, **kw)


def kernel(**inputs) -> np.ndarray:
    in_maps, fast = _host_prep(inputs)
    res = _run(in_maps, fast)
    return np.concatenate([res.results[m]["logits"] for m in range(NCORES)],
                          axis=0)


# revision 34
# speedup vs baseline: 1.0715x; 1.0715x over previous
"""Trainium2 Bass kernel for nn_MetaLearner (meta-learning attention + cosine
prototype scoring), data-parallel over tasks on 8 NeuronCores.

Math (per task):
  c   = [img, txt] @ Wc.T + bc                (Wc = concat(Wi, Wt))
  h   = LN1(c);  q,k,v = h @ W{q,k,v}.T + b   (queries: seqlen=1 -> ctx = v)
  ctx = softmax(q k^T / sqrt(128)) v          (support: seqlen=4)
  f   = LN2(ctx) @ Wo.T + bo
  logits[t,q,c] = 10 * cos(qf[t,q], sf[t,c])

Key tricks:
  - LN gains/biases and mean-subtractions folded into weights on host.
  - Softmax denominator (and max-subtraction) dropped: LN2's rstd cancels
    any positive per-column scale of ctx; mean-centering folded into Wv.
  - When bc/bv/bo fold to zero (true for the reference initialization),
    every per-column scale cancels through the final cosine normalize, so
    the whole query path collapses to qf = normalize(Wu x) with
    Wu = (Wo Wv_c) Wc_c folded on host -- queries never materialize c, h,
    v, or f; the projection accumulates directly during streaming.
    Support keeps LN1 (softmax is scale-sensitive) but skips LN2's rstd.
    A general fallback path keeps the full math.
  - rstd = Sqrt(reciprocal_approx_fast(scale * colsum(x^2))): the scale
    rides in the reduction weights, DVE does the reciprocal, ACT only ever
    evaluates Sqrt (plus one Exp for softmax) -> ~2 table switches total.
  - No transposes: attention scores computed pre-transposed (sT = kT^T qT)
    and v computed pre-transposed (vn = h^T WvT) by operand swapping.
  - All PE operands fp16 (1 cyc/row); f32 accumulation in PSUM.
  - Inputs streamed as fp16, host-packed per column-group so every stream
    DMA is one contiguous 1.4-5.6 MB transfer. The last group is only 256
    columns so the final (unhidable) tail chain is short.
On-chip layout is "transposed" throughout: activations are [hid, rows].
"""
import sys
sys.path.insert(0, "/opt/trn_rl_repo")
import numpy as np

HID = 128
T, Q, S = 256, 64, 4
DI, DTXT = 2048, 768
NCORES = 8
TPC = T // NCORES               # 32 tasks per core
FEAT = DI + DTXT                # 2816
KT = FEAT // 128                # 22 contraction chunks
QROWS = TPC * Q                 # 2048 query rows per core
SROWS = TPC * S                 # 128 support rows per core
ROWS = QROWS + SROWS            # 2176
SCALE_INV = 1.0 / (np.sqrt(HID) + 1e-8)

_progs = {}  # cached compiled Bass programs, keyed by fast-path flag


def _build(fast):
    import concourse.bacc as bacc
    import concourse.tile as tile
    import concourse.mybir as mybir
    import concourse.bass as _b

    F32 = mybir.dt.float32
    F16 = mybir.dt.float16
    AFT = mybir.ActivationFunctionType

    nc = bacc.Bacc()
    x1_d = nc.declare_dram_parameter("x1", [128, KT * 1152], F16, isOutput=False)
    x2_d = nc.declare_dram_parameter("x2", [128, KT * 512], F16, isOutput=False)
    x3a_d = nc.declare_dram_parameter("x3a", [128, KT * 320], F16,
                                      isOutput=False)
    x3b_d = nc.declare_dram_parameter("x3b", [128, KT * 192], F16,
                                      isOutput=False)
    wcwu_d = nc.declare_dram_parameter("wcwu", [128, KT * 256], F16,
                                       isOutput=False)
    cst_d = nc.declare_dram_parameter("consts", [128, 772], F16, isOutput=False)
    row0_d = nc.declare_dram_parameter("row0", [1, 768], F16, isOutput=False)
    bias_d = (None if fast else
              nc.declare_dram_parameter("biases", [HID, 5], F32,
                                        isOutput=False))
    out_d = nc.declare_dram_parameter("logits", [TPC, Q, S], F32, isOutput=True)

    lp = nc.allow_low_precision(reason="fp16 streaming with f32 accumulation")
    lp.__enter__()

    with tile.TileContext(nc) as tc:
        with (
            tc.tile_pool(name="wts", bufs=1) as wts,
            tc.tile_pool(name="xg1p", bufs=1) as xg1p,
            tc.tile_pool(name="xg2p", bufs=1) as xg2p,
            tc.tile_pool(name="xg3p", bufs=1) as xg3p,
            tc.tile_pool(name="qfp", bufs=1) as qfp,
            tc.tile_pool(name="wk", bufs=3) as wk,
            tc.tile_pool(name="pst", bufs=1, space="PSUM") as pst,
        ):
            # ---- loads, in stream order ----
            wcwu_t = wts.tile([128, KT, 256], F16)
            nc.sync.dma_start(out=wcwu_t, in_=wcwu_d[:])
            xg1_t = xg1p.tile([128, KT, 1152], F16)
            nc.sync.dma_start(out=xg1_t[:, 0:8, :], in_=x1_d[:, 0:8 * 1152])
            nc.sync.dma_start(out=xg1_t[:, 8:15, :],
                              in_=x1_d[:, 8 * 1152:15 * 1152])
            nc.sync.dma_start(out=xg1_t[:, 15:KT, :], in_=x1_d[:, 15 * 1152:])
            cst_t = wts.tile([128, 772], F16)
            nc.sync.dma_start(out=cst_t, in_=cst_d[:])
            wq_t = cst_t[:, 0 * HID:1 * HID]
            wk_t = cst_t[:, 1 * HID:2 * HID]
            wv_t = cst_t[:, 2 * HID:3 * HID]
            wo_t = cst_t[:, 3 * HID:4 * HID]
            wov_t = cst_t[:, 4 * HID:5 * HID]
            red_ln = cst_t[:, 640:641]           # 1/128
            red_q = cst_t[:, 641:642]            # 1.0
            red_s = cst_t[:, 642:643]            # 0.01
            mask_t = cst_t[:, 644:772]           # [128, 128] 0/1 block mask
            row0_t = wts.tile([1, 768], F16)
            nc.sync.dma_start(out=row0_t, in_=row0_d[:])
            ones_r = row0_t[:, 0:128]            # lhsT [K=1, M=128] for bcast
            ones_row = row0_t[:, 0:640]          # all-ones rhs
            bvrow_t = row0_t[:, 640:768]
            if not fast:
                bias_t = wts.tile([HID, 5], F32)
                nc.sync.dma_start(out=bias_t, in_=bias_d[:])
                bc_a, bq_a, bk_a, bv_a, bo_a = (
                    bias_t[:, i:i + 1] for i in range(5))
            else:
                bc_a = bq_a = bk_a = bv_a = bo_a = None
            xg2_t = xg2p.tile([128, KT, 512], F16)
            nc.sync.dma_start(out=xg2_t[:, 0:11, :], in_=x2_d[:, 0:11 * 512])
            nc.sync.dma_start(out=xg2_t[:, 11:KT, :], in_=x2_d[:, 11 * 512:])
            xg3a_t = xg3p.tile([128, KT, 320], F16)
            nc.sync.dma_start(out=xg3a_t[:, 0:11, :], in_=x3a_d[:, 0:11 * 320])
            nc.sync.dma_start(out=xg3a_t[:, 11:KT, :], in_=x3a_d[:, 11 * 320:])
            xg3b_t = xg3p.tile([128, KT, 192], F16)
            nc.sync.dma_start(out=xg3b_t[:, 0:11, :], in_=x3b_d[:, 0:11 * 192])
            nc.sync.dma_start(out=xg3b_t[:, 11:18, :],
                              in_=x3b_d[:, 11 * 192:18 * 192])
            nc.sync.dma_start(out=xg3b_t[:, 18:KT, :], in_=x3b_d[:, 18 * 192:])

            qf_t = qfp.tile([128, QROWS], F16)
            sf_t = qfp.tile([128, SROWS], F16)
            U_sb = qfp.tile([64, 2 * Q], F32)

            def rstd(sq_sb, cn, red):
                """1/sqrt(red . sq) broadcast to [128, cn] PSUM (f16 path)."""
                ss_ps = pst.tile([1, 512], F32, tag="ss", bufs=2)
                nc.tensor.matmul(ss_ps[:, :cn], red, sq_sb[:, :cn],
                                 start=True, stop=True)
                ir = wk.tile([1, 512], F32, tag="ir")
                nc.vector.reciprocal_approx_fast(out=ir[:, :cn],
                                                 in_=ss_ps[:, :cn])
                rr = wk.tile([1, 512], F16, tag="rr")
                nc.scalar.activation(out=rr[:, :cn], in_=ir[:, :cn],
                                     func=AFT.Sqrt, bias=0.0, scale=1.0)
                R_ps = pst.tile([128, 512], F32, tag="pp", bufs=2)
                nc.tensor.matmul(R_ps[:, :cn], ones_r, rr[:, :cn],
                                 start=True, stop=True)
                return R_ps

            def score(t0, nt):
                """U_sb[q, 4t:4(t+nt)] = qf[:,64t:...]^T sf[:,4t:...] x nt."""
                U_ps = pst.tile([64, 32], F32, tag="sc", bufs=1)
                for j in range(nt):
                    t = t0 + j
                    nc.tensor.matmul(U_ps[0:64, 4 * j:4 * j + 4],
                                     qf_t[:, 64 * t:64 * t + 64],
                                     sf_t[:, 4 * t:4 * t + 4],
                                     start=True, stop=True)
                nc.vector.tensor_copy(
                    out=U_sb[0:64, 4 * t0:4 * (t0 + nt)],
                    in_=U_ps[0:64, 0:4 * nt])

            def query_tail_fast(u_ps, qf_off, cn):
                """qf = normalize(u); u = Wu x accumulated during streaming."""
                ff = wk.tile([128, 512], F16, tag="ff")
                nc.vector.tensor_copy(out=ff[:, :cn], in_=u_ps[:, :cn])
                sq = wk.tile([128, 512], F16, tag="sq")
                nc.scalar.activation(out=sq[:, :cn], in_=u_ps[:, :cn],
                                     func=AFT.Square, bias=0.0, scale=1.0)
                RN = rstd(sq, cn, red_q)
                nc.vector.tensor_mul(out=qf_t[:, qf_off:qf_off + cn],
                                     in0=ff[:, :cn], in1=RN[:, :cn])
                score(qf_off // 64, cn // 64)

            def query_tail_gen(c_ps, qf_off, cn):
                cf = wk.tile([128, 512], F16, tag="cf")
                nc.vector.tensor_scalar_add(out=cf[:, :cn], in0=c_ps[:, :cn],
                                            scalar1=bc_a)
                sq = wk.tile([128, 512], F16, tag="sq")
                nc.scalar.activation(out=sq[:, :cn], in_=c_ps[:, :cn],
                                     func=AFT.Square, bias=bc_a, scale=1.0)
                R1 = rstd(sq, cn, red_ln)
                h = wk.tile([128, 512], F16, tag="h")
                nc.vector.tensor_mul(out=h[:, :cn], in0=cf[:, :cn],
                                     in1=R1[:, :cn])
                v_ps = pst.tile([128, 512], F32, tag="pp", bufs=2)
                nc.tensor.matmul(v_ps[:, :cn], wv_t, h[:, :cn],
                                 start=True, stop=True)
                vf = wk.tile([128, 512], F16, tag="vf")
                nc.vector.tensor_scalar_add(out=vf[:, :cn], in0=v_ps[:, :cn],
                                            scalar1=bv_a)
                sq2 = wk.tile([128, 512], F16, tag="sq")
                nc.scalar.activation(out=sq2[:, :cn], in_=v_ps[:, :cn],
                                     func=AFT.Square, bias=bv_a, scale=1.0)
                R2 = rstd(sq2, cn, red_ln)
                z = wk.tile([128, 512], F16, tag="h")
                nc.vector.tensor_mul(out=z[:, :cn], in0=vf[:, :cn],
                                     in1=R2[:, :cn])
                o_ps = pst.tile([128, 512], F32, tag="pp", bufs=2)
                nc.tensor.matmul(o_ps[:, :cn], wo_t, z[:, :cn],
                                 start=True, stop=True)
                ff = wk.tile([128, 512], F16, tag="ff")
                nc.vector.tensor_scalar_add(out=ff[:, :cn], in0=o_ps[:, :cn],
                                            scalar1=bo_a)
                sq3 = wk.tile([128, 512], F16, tag="sq")
                nc.scalar.activation(out=sq3[:, :cn], in_=o_ps[:, :cn],
                                     func=AFT.Square, bias=bo_a, scale=1.0)
                RN = rstd(sq3, cn, red_q)
                nc.vector.tensor_mul(out=qf_t[:, qf_off:qf_off + cn],
                                     in0=ff[:, :cn], in1=RN[:, :cn])
                score(qf_off // 64, cn // 64)

            query_tail = query_tail_fast if fast else query_tail_gen

            def support_tail(cS_ps):
                cfS = wk.tile([128, SROWS], F16, tag="cfS")
                if fast:
                    nc.vector.tensor_copy(out=cfS, in_=cS_ps[:, 0:SROWS])
                else:
                    nc.vector.tensor_scalar_add(out=cfS, in0=cS_ps[:, 0:SROWS],
                                                scalar1=bc_a)
                sqS = wk.tile([128, SROWS], F16, tag="sqS")
                if fast:
                    nc.scalar.activation(out=sqS, in_=cS_ps[:, 0:SROWS],
                                         func=AFT.Square, bias=0.0, scale=1.0)
                else:
                    nc.scalar.activation(out=sqS, in_=cS_ps[:, 0:SROWS],
                                         func=AFT.Square, bias=bc_a, scale=1.0)
                R1 = rstd(sqS, SROWS, red_ln)
                hs = wk.tile([128, SROWS], F16, tag="hs")
                nc.vector.tensor_mul(out=hs, in0=cfS, in1=R1[:, :SROWS])
                q_ps = pst.tile([128, 512], F32, tag="pp", bufs=2)
                nc.tensor.matmul(q_ps[:, :SROWS], wq_t, hs, start=True, stop=True)
                qTb = wk.tile([128, SROWS], F16, tag="qTb")
                if fast:
                    nc.vector.tensor_copy(out=qTb, in_=q_ps[:, :SROWS])
                else:
                    nc.vector.tensor_scalar_add(out=qTb, in0=q_ps[:, :SROWS],
                                                scalar1=bq_a)
                k_ps = pst.tile([128, 512], F32, tag="pp", bufs=2)
                nc.tensor.matmul(k_ps[:, :SROWS], wk_t, hs, start=True, stop=True)
                kTb = wk.tile([128, SROWS], F16, tag="kTb")
                if fast:
                    nc.vector.tensor_copy(out=kTb, in_=k_ps[:, :SROWS])
                else:
                    nc.vector.tensor_scalar_add(out=kTb, in0=k_ps[:, :SROWS],
                                                scalar1=bk_a)
                # vn = (Wv h)^T (+ 1 (x) bv in the general path) : [srows, hid]
                vn_ps = pst.tile([128, 512], F32, tag="pp", bufs=2)
                nc.tensor.matmul(vn_ps[:, :HID], hs, wv_t, start=True,
                                 stop=fast)
                if not fast:
                    nc.tensor.matmul(vn_ps[:, :HID], ones_row[:, 0:SROWS],
                                     bvrow_t, start=False, stop=True)
                vn_sb = wk.tile([SROWS, HID], F16, tag="vn")
                nc.vector.tensor_copy(out=vn_sb, in_=vn_ps[:, :HID])
                # sT[k_row, q_row] = k^T q ; unnormalized masked exp
                sT_ps = pst.tile([128, 512], F32, tag="pp", bufs=2)
                nc.tensor.matmul(sT_ps[:, :SROWS], kTb, qTb, start=True,
                                 stop=True)
                aT = wk.tile([SROWS, SROWS], F16, tag="aT")
                nc.scalar.activation(out=aT, in_=sT_ps[:, :SROWS], func=AFT.Exp,
                                     bias=0.0, scale=1.0)
                am = wk.tile([SROWS, SROWS], F16, tag="am")
                nc.vector.tensor_mul(out=am, in0=aT, in1=mask_t)
                ctx_ps = pst.tile([128, 512], F32, tag="pp", bufs=2)
                nc.tensor.matmul(ctx_ps[:, :SROWS], vn_sb, am, start=True,
                                 stop=True)
                cfx = wk.tile([128, SROWS], F16, tag="cfS")
                nc.vector.tensor_copy(out=cfx, in_=ctx_ps[:, :SROWS])
                if fast:
                    zx = cfx           # LN2 rstd cancels in the final normalize
                else:
                    sqx = wk.tile([128, SROWS], F16, tag="sqS")
                    nc.scalar.activation(out=sqx, in_=ctx_ps[:, :SROWS],
                                         func=AFT.Square, bias=0.0, scale=1.0)
                    R2 = rstd(sqx, SROWS, red_ln)
                    zx = wk.tile([128, SROWS], F16, tag="hs")
                    nc.vector.tensor_mul(out=zx, in0=cfx, in1=R2[:, :SROWS])
                o_ps = pst.tile([128, 512], F32, tag="pp", bufs=2)
                nc.tensor.matmul(o_ps[:, :SROWS], wo_t, zx, start=True, stop=True)
                ffx = wk.tile([128, SROWS], F16, tag="ffS")
                if fast:
                    nc.vector.tensor_copy(out=ffx, in_=o_ps[:, :SROWS])
                else:
                    nc.vector.tensor_scalar_add(out=ffx, in0=o_ps[:, :SROWS],
                                                scalar1=bo_a)
                sqf = wk.tile([128, SROWS], F16, tag="sqS")
                if fast:
                    nc.scalar.activation(out=sqf, in_=o_ps[:, :SROWS],
                                         func=AFT.Square, bias=0.0, scale=1.0)
                else:
                    nc.scalar.activation(out=sqf, in_=o_ps[:, :SROWS],
                                         func=AFT.Square, bias=bo_a, scale=1.0)
                RN = rstd(sqf, SROWS, red_s)   # = 10/||f||
                nc.vector.tensor_mul(out=sf_t[:], in0=ffx, in1=RN[:, :SROWS])

            def qw_k(k):
                return (wcwu_t[:, k, 128:256] if fast
                        else wcwu_t[:, k, 0:128])

            def out_dma(t0, nt):
                dst = _b.AP(tensor=out_d[:].tensor,
                            offset=out_d[:].offset + 256 * t0,
                            ap=[[4, 64], [256, nt], [1, 4]])
                nc.sync.dma_start(
                    out=dst,
                    in_=U_sb[0:64, 4 * t0:4 * (t0 + nt)].rearrange(
                        "p (g b) -> p g b", b=4))

            # ---- group 1: support + q0 + q1 ----
            with tc.tile_pool(name="psA", bufs=1, space="PSUM") as psA:
                cS_ps = psA.tile([128, SROWS], F32)
                u0_ps = psA.tile([128, 512], F32)
                u1_ps = psA.tile([128, 512], F32)
                for k in range(KT):
                    nc.tensor.matmul(cS_ps[:], wcwu_t[:, k, 0:128],
                                     xg1_t[:, k, 0:128],
                                     start=(k == 0), stop=(k == KT - 1))
                    nc.tensor.matmul(u0_ps[:], qw_k(k),
                                     xg1_t[:, k, 128:640],
                                     start=(k == 0), stop=(k == KT - 1))
                    nc.tensor.matmul(u1_ps[:], qw_k(k),
                                     xg1_t[:, k, 640:1152],
                                     start=(k == 0), stop=(k == KT - 1))
                support_tail(cS_ps)
                query_tail(u0_ps, 0, 512)
                query_tail(u1_ps, 512, 512)
                out_dma(0, 16)

            # ---- groups 2+3: all stream matmuls back-to-back on PE,
            # then the latency-chained tails (keeps HAM warm at stream end)
            with tc.tile_pool(name="psB", bufs=1, space="PSUM") as psB:
                u2_ps = psB.tile([128, 512], F32)
                for k in range(KT):
                    nc.tensor.matmul(u2_ps[:], qw_k(k), xg2_t[:, k, :],
                                     start=(k == 0), stop=(k == KT - 1))
                with tc.tile_pool(name="psC", bufs=1, space="PSUM") as psC:
                    ua_ps = psC.tile([128, 320], F32)
                    ub_ps = psC.tile([128, 192], F32)
                    for k in range(KT):
                        nc.tensor.matmul(ua_ps[:], qw_k(k), xg3a_t[:, k, :],
                                         start=(k == 0), stop=(k == KT - 1))
                    for k in range(KT):
                        nc.tensor.matmul(ub_ps[:], qw_k(k), xg3b_t[:, k, :],
                                         start=(k == 0), stop=(k == KT - 1))
                    query_tail(u2_ps, 1024, 512)
                    out_dma(16, 8)
                    query_tail(ua_ps, 1536, 320)
                    out_dma(24, 5)
                    query_tail(ub_ps, 1856, 192)
                    out_dma(29, 3)

    lp.__exit__(None, None, None)
    nc.compile()
    return nc


def _host_prep(inputs):
    f32, f16 = np.float32, np.float16
    Wi, Wt = np.asarray(inputs["Wi"], f32), np.asarray(inputs["Wt"], f32)
    bi, bt = np.asarray(inputs["bi"], f32), np.asarray(inputs["bt"], f32)
    g1, b1 = np.asarray(inputs["g1"], f32), np.asarray(inputs["b1"], f32)
    g2, b2 = np.asarray(inputs["g2"], f32), np.asarray(inputs["b2"], f32)
    Wq, bq = np.asarray(inputs["Wq"], f32), np.asarray(inputs["bq"], f32)
    Wk, bk = np.asarray(inputs["Wk"], f32), np.asarray(inputs["bk"], f32)
    Wv, bv = np.asarray(inputs["Wv"], f32), np.asarray(inputs["bv"], f32)
    Wo, bo = np.asarray(inputs["Wo"], f32), np.asarray(inputs["bo"], f32)

    Wc = np.concatenate([Wi, Wt], axis=1)          # [128, 2816]
    bc = bi + bt
    Wc_c = Wc - Wc.mean(axis=0, keepdims=True)     # fold LN1 mean
    bc_c = bc - bc.mean()

    Wq_f = (Wq * g1[None, :]) * SCALE_INV
    bq_f = (bq + Wq @ b1) * SCALE_INV
    Wk_f = Wk * g1[None, :]
    bk_f = bk + Wk @ b1
    Wv_f = Wv * g1[None, :]
    bv_f = bv + Wv @ b1
    Wv_c = Wv_f - Wv_f.mean(axis=0, keepdims=True)  # fold LN2 mean
    bv_c = bv_f - bv_f.mean()
    Wo_f = Wo * g2[None, :]
    bo_f = bo + Wo @ b2
    Wov = Wo_f @ Wv_c                               # combined v+o projection
    Wu = Wov @ Wc_c                                 # full query-path fold

    fast = all(np.abs(b).max() < 1e-12
               for b in (bc_c, bv_c, bo_f, bq_f, bk_f))

    blk = np.arange(SROWS) // S
    mask01 = (blk[:, None] == blk[None, :]).astype(f16)
    consts = np.zeros((128, 772), f16)
    consts[:, 0:640] = np.concatenate(
        [Wq_f.T, Wk_f.T, Wv_c.T, Wo_f.T, Wov.T], axis=1).astype(f16)
    consts[:, 640] = f16(1.0 / HID)
    consts[:, 641] = f16(1.0)
    consts[:, 642] = f16(0.01)
    consts[:, 644:772] = mask01
    row0 = np.zeros((1, 768), f16)
    row0[0, 0:640] = 1.0
    row0[0, 640:768] = bv_c.astype(f16)

    def pack_kmajor(a):   # [feat, cols] -> [128, KT*cols] (p, k, c)
        cols = a.shape[1]
        return np.ascontiguousarray(
            a.reshape(KT, 128, cols).transpose(1, 0, 2).reshape(128, -1)
        )

    wcwu = np.concatenate([Wc_c.T.astype(f16).reshape(KT, 128, 128),
                           Wu.T.astype(f16).reshape(KT, 128, 128)],
                          axis=2)          # [KT, 128, 256] (wc_k | wu_k)
    common = {
        "wcwu": np.ascontiguousarray(
            wcwu.transpose(1, 0, 2).reshape(128, -1)),
        "consts": consts,
        "row0": row0,
    }
    if not fast:
        common["biases"] = np.ascontiguousarray(
            np.stack([bc_c, bq_f, bk_f, bv_c, bo_f], axis=1))

    si = np.asarray(inputs["support_images"], f32)
    st = np.asarray(inputs["support_texts"], f32)
    qi = np.asarray(inputs["query_images"], f32)
    qt = np.asarray(inputs["query_texts"], f32)

    in_maps = []
    for m in range(NCORES):
        ts = slice(m * TPC, (m + 1) * TPC)
        Xq = np.concatenate([qi[ts].reshape(QROWS, DI),
                             qt[ts].reshape(QROWS, DTXT)], axis=1)
        Xs = np.concatenate([si[ts].reshape(SROWS, DI),
                             st[ts].reshape(SROWS, DTXT)], axis=1)
        xT = np.concatenate([Xs, Xq], axis=0).T.astype(f16)  # [2816, 2176]
        in_maps.append({
            "x1": pack_kmajor(xT[:, 0:1152]),
            "x2": pack_kmajor(xT[:, 1152:1664]),
            "x3a": pack_kmajor(xT[:, 1664:1984]),
            "x3b": pack_kmajor(xT[:, 1984:2176]),
            **common,
        })
    return in_maps, fast


def _run(in_maps, fast, trace=False, **kw):
    from concourse.bass_utils import run_bass_kernel_spmd
    if fast not in _progs:
        _progs[fast] = _build(fast)
    return run_bass_kernel_spmd(_progs[fast], in_maps, list(range(NCORES)),
                                trace=trace, **kw)


def kernel(**inputs) -> np.ndarray:
    in_maps, fast = _host_prep(inputs)
    res = _run(in_maps, fast)
    return np.concatenate([res.results[m]["logits"] for m in range(NCORES)],
                          axis=0)
